# revision 1
# baseline (speedup 1.0000x reference)
"""DiagonalBiLSTM Trainium2 kernel.

Full inputs in, full output out. Internally: 8-way data-parallel over the
1024 flattened (batch, height) scan rows; both scan directions fused into
one moving dimension (N=508) so fp32r matmuls run at full rate. The
cross-core h_next row coupling is handled with 126 redundant ghost rows
per core (no inter-core communication).

Math per diagonal step d (per row r, channel vector form):
    u  = Wm @ x_diag[d] + k0 @ h[r] + k1 @ h[r+1] + (b_i2s + b_s2s)
    g  = w_ih @ u + (b_ih + b_hh)          # 4*256 gate channels
    c  = sig(g_f)*c + sig(g_i)*tanh(g_g)
    h  = sig(g_o)*tanh(c)
"""

import numpy as np

B, CIN, COUT, H, W, DC = 16, 256, 256, 64, 64, 3
WD = 2 * W - 1          # 127 diagonal steps
NCORES = 8
OWN = 128               # own rows per core (2 batches)
GHOST = 126             # redundant ghost rows
NDATA = OWN + GHOST     # 254 data cols per direction
NCOL = 256              # padded col count per dir (col 254 = always zero)
NKC = 2                 # channel chunks (256 = 2*128)
NGT = 8                 # gate m-tiles (1024 = 8*128)

_COMPILED = {}


# ----------------------------------------------------------------- host prep

def _i2s_mask_np():
    oc = np.arange(COUT) % DC
    ic = np.arange(CIN) % DC
    return (ic[None, :] <= oc[:, None]).astype(np.float32)


def _wT_tiles(w, nmt):
    # [out=nmt*128, in=256] -> lhsT tile array [k=128, kc=2, mt=nmt, m=128]
    return np.ascontiguousarray(
        w.T.reshape(NKC, 128, nmt, 128).transpose(1, 0, 2, 3))


def _diag_pack(x_loc):
    """x_loc [4, 256, 64, 64] (local batches, already W-flipped for the R dir)
    -> [WD, NKC, 128, 256cols] where col j = local row (b_loc*64 + h),
    value x[b, c, h, d - h] (0 outside the diagonal band)."""
    xs = np.zeros((WD, 4, CIN, H), np.float32)
    for h in range(H):
        # diag d = h + w for w in 0..63  ->  xs[h:h+64, :, :, h]
        xs[h:h + W, :, :, h] = x_loc[:, :, h, :].transpose(2, 0, 1)
    # [WD, 4b, 256c, 64h] -> [WD, 256c, 4b*64h] -> [WD, 2, 128, 256]
    xs = xs.transpose(0, 2, 1, 3).reshape(WD, CIN, 4 * H)
    return np.ascontiguousarray(xs.reshape(WD, NKC, 128, 4 * H))


def _prep_inputs(x, w_i2s, b_i2s, w_ih, b_ih, b_hh, k0, k1, b_s2s):
    wm = w_i2s * _i2s_mask_np()
    wm_t = _wT_tiles(wm, 2)
    k0_t = _wT_tiles(k0, 2)
    k1_t = _wT_tiles(k1, 2)
    wih_t = _wT_tiles(w_ih, 8)

    bias_u = (b_i2s + b_s2s).astype(np.float32)       # [256]
    bias_g = (b_ih + b_hh).astype(np.float32)         # [1024]
    misc_all = np.zeros((NCORES, 128, 12), np.float32)
    misc_all[:, :, 0:2] = bias_u.reshape(2, 128).T[None]
    misc_all[:, :, 2:10] = bias_g.reshape(8, 128).T[None]
    misc_all[:, :, 10] = 1.0
    misc_all[7, :, 10] = 0.0                          # core 7: zero ghost0 h

    xf = x[:, :, :, ::-1]                             # W-flip for R direction
    in_maps = []
    for c in range(NCORES):
        xloc = np.zeros((4, CIN, H, W), np.float32)
        xfloc = np.zeros((4, CIN, H, W), np.float32)
        nb = min(4, B - 2 * c)
        xloc[:nb] = x[2 * c:2 * c + nb]
        xfloc[:nb] = xf[2 * c:2 * c + nb]
        dl = _diag_pack(xloc)                         # [WD, 2, 128, 256]
        dr = _diag_pack(xfloc)
        xd = np.stack([dl, dr], axis=3)               # [WD, 2, 128, 2, 256]
        in_maps.append({
            "xd": np.ascontiguousarray(xd),
            "wm": wm_t, "k0t": k0_t, "k1t": k1_t, "wih": wih_t,
            "misc": misc_all[c],
        })
    return in_maps


# ------------------------------------------------------- reference-free host
# numpy replica of the device program, for debugging (same per-core arrays)

def _core_sim(im, nsteps=WD):
    xd = im["xd"]                   # [WD, 2, 128, 2, 256]
    wm_t, k0_t, k1_t, wih_t = im["wm"], im["k0t"], im["k1t"], im["wih"]
    misc = im["misc"]

    def unT(t, nmt):                # tile array -> [out, in]
        return t.transpose(1, 0, 2, 3).reshape(CIN, nmt * 128).T

    wm, k0, k1, wih = unT(wm_t, 2), unT(k0_t, 2), unT(k1_t, 2), unT(wih_t, 8)
    bias_u = misc[:, 0:2].T.reshape(CIN)
    bias_g = misc[:, 2:10].T.reshape(8 * 128)
    s = misc[0, 10]

    def sig(v):
        return 1.0 / (1.0 + np.exp(-v))

    h = np.zeros((CIN, 2 * NCOL), np.float32)    # [ch, dir*col]
    hv = h.reshape(CIN, 2, NCOL)
    cst = np.zeros((CIN, 2 * NDATA), np.float32)
    out = np.zeros((WD, 2, 128, 2, OWN), np.float32)
    hp = np.empty((CIN, 2, NDATA), np.float32)
    hn = np.empty((CIN, 2, NDATA), np.float32)
    for d in range(nsteps):
        xs = np.ascontiguousarray(
            xd[d].reshape(CIN, 2, NCOL)[:, :, :NDATA]).reshape(CIN, -1)
        hp[:] = hv[:, :, 0:NDATA]
        hn[:] = hv[:, :, 1:NDATA + 1]
        u = (wm @ xs + k0 @ hp.reshape(CIN, -1) + k1 @ hn.reshape(CIN, -1)
             + bias_u[:, None])
        g = wih @ u + bias_g[:, None]
        gi, gf, gg, go = g[0:256], g[256:512], g[512:768], g[768:1024]
        cst = sig(gf) * cst + sig(gi) * np.tanh(gg)
        hv[:, :, 0:NDATA] = (sig(go) * np.tanh(cst)).reshape(CIN, 2, NDATA)
        hv[:, :, 128] *= s
        out[d] = hv.reshape(2, 128, 2, NCOL)[:, :, :, 0:OWN]
    return out


# ----------------------------------------------------------- output assembly

def _assemble(core_outs):
    # core_outs: list of [WD, 2, 128, 2, OWN] -> hs [2dir, WD, 256ch, 1024rows]
    hs = np.zeros((2, WD, CIN, B * H), np.float32)
    for c, o in enumerate(core_outs):
        o = np.asarray(o)
        hs[:, :, :, c * OWN:(c + 1) * OWN] = (
            o.transpose(3, 0, 1, 2, 4).reshape(2, WD, CIN, OWN))

    def unscramble(hd):             # [WD, 256ch, 1024rows] -> [B, COUT, H, WD]
        a = hd.transpose(0, 2, 1).reshape(WD, B, COUT, H)
        return a.transpose(1, 2, 3, 0)

    def unshift(a):                 # [B, COUT, H, WD] -> [B, COUT, H, W]
        rows = np.arange(H)[:, None]
        cols = rows + np.arange(W)[None, :]
        return a[:, :, rows, cols]

    left = unshift(unscramble(hs[0]))
    right = unshift(unscramble(hs[1]))[:, :, :, ::-1]
    right = np.concatenate(
        [np.zeros_like(right[:, :, :1, :]), right[:, :, :-1, :]], axis=2)
    return left + right


# --------------------------------------------------------------- bass kernel

def _build(nsteps=WD):
    import concourse.bacc as bacc
    import concourse.mybir as mybir
    import concourse.tile as tile
    from concourse._compat import get_trn_type

    f32 = mybir.dt.float32
    f32r = mybir.dt.float32r
    AF = mybir.ActivationFunctionType

    nc = bacc.Bacc(get_trn_type() or "TRN2", target_bir_lowering=False,
                   debug=False)
    xd = nc.dram_tensor("xd", [WD, NKC, 128, 2, NCOL], f32r,
                        kind="ExternalInput")
    wm = nc.dram_tensor("wm", [128, NKC, 2, 128], f32r, kind="ExternalInput")
    k0t = nc.dram_tensor("k0t", [128, NKC, 2, 128], f32r, kind="ExternalInput")
    k1t = nc.dram_tensor("k1t", [128, NKC, 2, 128], f32r, kind="ExternalInput")
    wih = nc.dram_tensor("wih", [128, NKC, NGT, 128], f32r,
                         kind="ExternalInput")
    misc = nc.dram_tensor("misc", [128, 12], f32, kind="ExternalInput")
    hs_out = nc.dram_tensor("hs", [WD, NKC, 128, 2, OWN], f32r,
                            kind="ExternalOutput")

    with tile.TileContext(nc) as tc:
        with (
            tc.tile_pool(name="wpool", bufs=1) as wpool,
            tc.tile_pool(name="state", bufs=1) as state,
            tc.tile_pool(name="xpool", bufs=3) as xpool,
            tc.tile_pool(name="upool", bufs=2) as upool,
            tc.tile_pool(name="apool", bufs=2) as apool,
            tc.tile_pool(name="tpool", bufs=2) as tpool,
            tc.tile_pool(name="upsum", bufs=2, space="PSUM") as upsum,
            tc.tile_pool(name="gpsum", bufs=5, space="PSUM") as gpsum,
        ):
            wm_t = wpool.tile([128, NKC, 2, 128], f32r, tag="wm")
            k0_t = wpool.tile([128, NKC, 2, 128], f32r, tag="k0")
            k1_t = wpool.tile([128, NKC, 2, 128], f32r, tag="k1")
            wih_t = wpool.tile([128, NKC, NGT, 128], f32r, tag="wih")
            misc_t = wpool.tile([128, 12], f32, tag="misc")
            nc.sync.dma_start(wm_t[:], wm[:])
            nc.sync.dma_start(k0_t[:], k0t[:])
            nc.sync.dma_start(k1_t[:], k1t[:])
            nc.sync.dma_start(wih_t[:], wih[:])
            nc.sync.dma_start(misc_t[:], misc[:])

            h = state.tile([128, NKC, 2, NCOL], f32r, tag="h")
            cs = state.tile([128, NKC, 2, NDATA], f32, tag="c")
            nc.any.memset(h[:].bitcast(f32), 0.0)
            nc.any.memset(cs[:], 0.0)

            for d in range(nsteps):
                xs = xpool.tile([128, NKC, 2, NCOL], f32r, tag="xs")
                for kc in range(NKC):
                    nc.sync.dma_start(xs[:, kc], xd[d, kc])

                u = upool.tile([128, NKC, 2, NDATA], f32r, tag="u")
                for m in range(NKC):
                    up = upsum.tile([128, 2, NDATA], f32, tag="up")
                    for kc in range(NKC):
                        nc.tensor.matmul(
                            up[:], wm_t[:, kc, m, :],
                            xs[:, kc, :, 0:NDATA],
                            start=(kc == 0), stop=False)
                    for kc in range(NKC):
                        nc.tensor.matmul(
                            up[:], k0_t[:, kc, m, :],
                            h[:, kc, :, 0:NDATA], start=False, stop=False)
                    for kc in range(NKC):
                        nc.tensor.matmul(
                            up[:], k1_t[:, kc, m, :],
                            h[:, kc, :, 1:NDATA + 1],
                            start=False, stop=(kc == NKC - 1))
                    nc.vector.tensor_scalar_add(
                        u[:, m], up[:], misc_t[:, m:m + 1])

                acts = []
                for t in range(NGT):
                    gp = gpsum.tile([128, 2, NDATA], f32, tag="gp")
                    for kc in range(NKC):
                        nc.tensor.matmul(
                            gp[:], wih_t[:, kc, t, :], u[:, kc],
                            start=(kc == 0), stop=(kc == NKC - 1))
                    a = apool.tile([128, 2, NDATA], f32, tag=f"act{t}")
                    fn = AF.Tanh if t in (4, 5) else AF.Sigmoid
                    nc.scalar.activation(a[:], gp[:], fn,
                                         bias=misc_t[:, 2 + t:3 + t])
                    acts.append(a)

                for m in range(NKC):
                    t1 = tpool.tile([128, 2, NDATA], f32, tag=f"t1_{m}")
                    nc.vector.tensor_mul(t1[:], acts[0 + m][:], acts[4 + m][:])
                    nc.vector.tensor_mul(cs[:, m], cs[:, m], acts[2 + m][:])
                    nc.vector.tensor_add(cs[:, m], cs[:, m], t1[:])
                    t2 = tpool.tile([128, 2, NDATA], f32, tag=f"t2_{m}")
                    nc.scalar.activation(t2[:], cs[:, m], AF.Tanh)
                    nc.vector.tensor_mul(h[:, m, :, 0:NDATA], acts[6 + m][:],
                                         t2[:])
                nc.vector.tensor_scalar_mul(
                    h[:, :, :, OWN:OWN + 1], h[:, :, :, OWN:OWN + 1],
                    misc_t[:, 10:11])

                for kc in range(NKC):
                    nc.sync.dma_start(hs_out[d, kc], h[:, kc, :, 0:OWN])

    nc.finalize()
    return nc


def _get_compiled(nsteps=WD):
    if nsteps not in _COMPILED:
        _COMPILED[nsteps] = _build(nsteps)
    return _COMPILED[nsteps]


# ------------------------------------------------------------------- driver

def kernel(x, w_i2s, b_i2s, w_ih, b_ih, b_hh, k0, k1, b_s2s):
    from concourse.bass_utils import run_bass_kernel_spmd

    in_maps = _prep_inputs(np.asarray(x, np.float32), np.asarray(w_i2s),
                           np.asarray(b_i2s), np.asarray(w_ih),
                           np.asarray(b_ih), np.asarray(b_hh),
                           np.asarray(k0), np.asarray(k1), np.asarray(b_s2s))
    nc = _get_compiled()
    res = run_bass_kernel_spmd(nc, in_maps, list(range(NCORES)))
    return _assemble([res.results[c]["hs"] for c in range(NCORES)])


def kernel_numpy(x, w_i2s, b_i2s, w_ih, b_ih, b_hh, k0, k1, b_s2s):
    """Host-only replica of the device program (debug path)."""
    in_maps = _prep_inputs(np.asarray(x, np.float32), np.asarray(w_i2s),
                           np.asarray(b_i2s), np.asarray(w_ih),
                           np.asarray(b_ih), np.asarray(b_hh),
                           np.asarray(k0), np.asarray(k1), np.asarray(b_s2s))
    return _assemble([_core_sim(im) for im in in_maps])



# revision 13
# speedup vs baseline: 1.3671x; 1.3671x over previous
"""DiagonalBiLSTM Trainium2 kernel, v2.

Full inputs in, full output out. 8-way data-parallel over the 1024
flattened (batch, height) rows per direction; 126 redundant ghost rows
per core, but the computed width SHRINKS each step (W(d) = 254 - d)
tracking the light cone of the h[r+1] coupling, so the average width is
191 instead of 254.

All matmul operands are bf16 (full PE rate at any moving size + FWL),
PSUM accumulation fp32, cell state bf16 (validated: final rel err ~6e-3).
The input conv xw = (Wm*mask)@x + b_i2s + b_s2s is precomputed on the
host, and the gate bias (b_ih + b_hh) is applied via the activation
instruction's per-partition bias operand.

Math per diagonal step d (per row r, channel vector form):
    u  = xw[d] + k0 @ h[r] + k1 @ h[r+1]
    g  = w_ih @ u                          # + bias via activation
    c  = sig(g_f)*c + sig(g_i)*tanh(g_g)
    h  = sig(g_o)*tanh(c)
"""

import numpy as np
import ml_dtypes

BF16 = ml_dtypes.bfloat16

B, CIN, COUT, H, W, DC = 16, 256, 256, 64, 64, 3
WD = 2 * W - 1          # 127 diagonal steps
NCORES = 8
OWN = 128               # own rows per core (2 batches)
ND = 254                # max data cols per direction (own + 126 ghost)
NKC = 2                 # channel chunks (256 = 2*128)
NGT = 8                 # gate m-tiles (1024 = 8*128)

_COMPILED = {}


# ----------------------------------------------------------------- host prep

def _i2s_mask_np():
    oc = np.arange(COUT) % DC
    ic = np.arange(CIN) % DC
    return (ic[None, :] <= oc[:, None]).astype(np.float32)


def _wT_tiles(w, nmt):
    # [out=nmt*128, in=256] -> lhsT tile array [k=128, kc=2, mt=nmt, m=128]
    return np.ascontiguousarray(
        w.T.reshape(NKC, 128, nmt, 128).transpose(1, 0, 2, 3)).astype(BF16)


def _diag_pack(x_loc):
    """x_loc [4, 256, 64, 64] (local batches, already W-flipped for the R
    dir) -> [WD, NKC, 128, 256cols], col j = local row (b_loc*64 + h)."""
    xs = np.zeros((WD, 4, CIN, H), np.float32)
    for h in range(H):
        xs[h:h + W, :, :, h] = x_loc[:, :, h, :].transpose(2, 0, 1)
    xs = xs.transpose(0, 2, 1, 3).reshape(WD, CIN, 4 * H)
    return np.ascontiguousarray(xs.reshape(WD, NKC, 128, 4 * H))


def _prep_inputs(x, w_i2s, b_i2s, w_ih, b_ih, b_hh, k0, k1, b_s2s):
    x = np.asarray(x, np.float32)
    wm = (np.asarray(w_i2s, np.float32) * _i2s_mask_np())
    # host input conv: xw[o, b, h, w] = sum_c wm[o,c] x[b,c,h,w] + biases
    xw = np.tensordot(wm, x, axes=([1], [1]))            # [256, B, H, W]
    xw = np.ascontiguousarray(xw.transpose(1, 0, 2, 3))  # [B, 256, H, W]
    # bias applies on the SKEWED map: every (step, row) position gets it,
    # including outside the diagonal band (pre/post-band warm-up of the
    # recurrence), so it is added after packing, not here.
    bias_u = (np.asarray(b_i2s) + np.asarray(b_s2s)).astype(
        np.float32).reshape(NKC, 128)

    k0_t = _wT_tiles(np.asarray(k0, np.float32), 2)
    k1_t = _wT_tiles(np.asarray(k1, np.float32), 2)
    wih_t = _wT_tiles(np.asarray(w_ih, np.float32), 8)

    bias_g = (np.asarray(b_ih) + np.asarray(b_hh)).astype(np.float32)
    misc_all = np.zeros((NCORES, 128, 12), np.float32)
    misc_all[:, :, 2:10] = bias_g.reshape(8, 128).T[None]
    misc_all[:, :, 10] = 1.0
    misc_all[7, :, 10] = 0.0                  # core 7: zero h at col OWN

    xf = xw[:, :, :, ::-1]                    # W-flip for R direction
    in_maps = []
    for c in range(NCORES):
        xloc = np.zeros((4, CIN, H, W), np.float32)
        xfloc = np.zeros((4, CIN, H, W), np.float32)
        nb = min(4, B - 2 * c)
        xloc[:nb] = xw[2 * c:2 * c + nb]
        xfloc[:nb] = xf[2 * c:2 * c + nb]
        dl = _diag_pack(xloc)                 # [WD, 2, 128, 256]
        dr = _diag_pack(xfloc)
        xd = np.stack([dl, dr], axis=3)[:, :, :, :, :ND]   # [WD,2,128,2,254]
        xd = xd + bias_u[None, :, :, None, None]
        in_maps.append({
            "xd": np.ascontiguousarray(xd).astype(BF16),
            "k0t": k0_t, "k1t": k1_t, "wih": wih_t,
            "misc": misc_all[c],
        })
    return in_maps


# ----------------------------------------------------------- output assembly

def _assemble(core_outs):
    # core_outs: list of [WD, 128, 2kc, 2dir, OWN]
    hs = np.zeros((2, WD, CIN, B * H), np.float32)
    for c, o in enumerate(core_outs):
        a = np.asarray(o).astype(np.float32)
        # [WD, p, kc, dir, j] -> [dir, WD, kc, p, j] -> [dir, WD, 256, OWN]
        a = a.transpose(3, 0, 2, 1, 4).reshape(2, WD, CIN, OWN)
        hs[:, :, :, c * OWN:(c + 1) * OWN] = a

    def unscramble(hd):             # [WD, 256ch, 1024rows] -> [B,COUT,H,WD]
        a = hd.transpose(0, 2, 1).reshape(WD, B, COUT, H)
        return a.transpose(1, 2, 3, 0)

    def unshift(a):                 # [B, COUT, H, WD] -> [B, COUT, H, W]
        rows = np.arange(H)[:, None]
        cols = rows + np.arange(W)[None, :]
        return a[:, :, rows, cols]

    left = unshift(unscramble(hs[0]))
    right = unshift(unscramble(hs[1]))[:, :, :, ::-1]
    right = np.concatenate(
        [np.zeros_like(right[:, :, :1, :]), right[:, :, :-1, :]], axis=2)
    return left + right


# --------------------------------------------------------------- bass kernel

def _build(nsteps=WD):
    import concourse.bacc as bacc
    import concourse.mybir as mybir
    import concourse.tile as tile
    from concourse._compat import get_trn_type

    f32 = mybir.dt.float32
    bf = mybir.dt.bfloat16
    AF = mybir.ActivationFunctionType

    nc = bacc.Bacc(get_trn_type() or "TRN2", target_bir_lowering=False,
                   debug=False)
    xd = nc.dram_tensor("xd", [WD, NKC, 128, 2, ND], bf, kind="ExternalInput")
    k0t = nc.dram_tensor("k0t", [128, NKC, 2, 128], bf, kind="ExternalInput")
    k1t = nc.dram_tensor("k1t", [128, NKC, 2, 128], bf, kind="ExternalInput")
    wih = nc.dram_tensor("wih", [128, NKC, NGT, 128], bf,
                         kind="ExternalInput")
    misc = nc.dram_tensor("misc", [128, 12], f32, kind="ExternalInput")
    hs_out = nc.dram_tensor("hs", [WD, 128, NKC, 2, OWN], bf,
                            kind="ExternalOutput")

    # gate m-tile order within the shared 8-bank PSUM tile:
    # bank 0: i(kc0)   1: i(kc1)   2: f0  3: f1  4: g0  5: g1  6: o0  7: o1
    with tile.TileContext(nc) as tc:
        with (
            tc.tile_pool(name="wpool", bufs=1) as wpool,
            tc.tile_pool(name="state", bufs=1) as state,
            tc.tile_pool(name="xpool", bufs=3) as xpool,
            tc.tile_pool(name="upool", bufs=2) as upool,
            tc.tile_pool(name="apool", bufs=2) as apool,
            tc.tile_pool(name="tpool", bufs=2) as tpool,
            tc.tile_pool(name="psum", bufs=1, space="PSUM") as psum,
        ):
            k0_t = wpool.tile([128, NKC, 2, 128], bf, tag="k0")
            k1_t = wpool.tile([128, NKC, 2, 128], bf, tag="k1")
            wih_t = wpool.tile([128, NKC, NGT, 128], bf, tag="wih")
            misc_t = wpool.tile([128, 12], f32, tag="misc")
            nc.sync.dma_start(k0_t[:], k0t[:])
            nc.sync.dma_start(k1_t[:], k1t[:])
            nc.sync.dma_start(wih_t[:], wih[:])
            nc.sync.dma_start(misc_t[:], misc[:])

            # h/c state: [128, kc, dir, col]; col ND..255 stays zero
            h = state.tile([128, NKC, 2, 256], bf, tag="h")
            cs = state.tile([128, NKC, 2, ND], bf, tag="c")
            nc.any.memset(h[:], 0.0)
            nc.any.memset(cs[:], 0.0)

            # one 8-bank PSUM tile shared by u (banks 0-1, early in the
            # step) and the 8 gate m-tiles (banks 0-7, later).  Within a
            # bank: [0:Wd] = dir L, [256:256+Wd] = dir R.
            pt = psum.tile([128, 8, 2, 256], f32, tag="pt")

            for d in range(nsteps):
                Wd = ND - d if d < ND - OWN else OWN   # valid width
                xs = xpool.tile([128, NKC, 2, ND], bf, tag="xs")
                for kc in range(NKC):
                    nc.sync.dma_start(xs[:, kc, :, 0:Wd],
                                      xd[d, kc, :, :, 0:Wd])

                # u = k0 @ h + k1 @ h(+1)   (banks 0-1)
                # NOTE: two accumulation groups share each PSUM bank (dir
                # L at [0:Wd], dir R at [256:256+Wd]).  Only the FIRST
                # group may use start=True — start clears the has_written
                # bits of the whole bank, which would corrupt the other
                # group.  The second group relies on overwrite-where-
                # unwritten semantics instead.
                for m in range(NKC):
                    for dr in range(2):
                        dst = pt[:, m, dr, 0:Wd]
                        first = (dr == 0)
                        for wt, sh in ((k0_t, 0), (k1_t, 1)):
                            for kc in range(NKC):
                                nc.tensor.matmul(
                                    dst, wt[:, kc, m, :],
                                    h[:, kc, dr, sh:sh + Wd],
                                    start=first,
                                    stop=(wt is k1_t and kc == NKC - 1),
                                    skip_group_check=True)
                                first = False

                u = upool.tile([128, NKC, 2, ND], bf, tag="u")
                for m in range(NKC):
                    nc.vector.tensor_add(
                        u[:, m, :, 0:Wd], pt[:, m, :, 0:Wd],
                        xs[:, m, :, 0:Wd])

                # gates (banks 0-7) + activations; emit m=0's four gates
                # (i0,f0,g0,o0) first so the m=0 cell chain overlaps m=1
                # gate matmuls/activations
                acts = [None] * NGT
                for m in range(NKC):
                    for t in (0 + m, 2 + m, 4 + m, 6 + m):
                        for dr in range(2):
                            for kc in range(NKC):
                                nc.tensor.matmul(
                                    pt[:, t, dr, 0:Wd], wih_t[:, kc, t, :],
                                    u[:, kc, dr, 0:Wd],
                                    start=(dr == 0 and kc == 0),
                                    stop=(kc == NKC - 1),
                                    skip_group_check=True)
                        a = apool.tile([128, 2, ND], bf, tag=f"act{t}")
                        fn = AF.Tanh if t in (4, 5) else AF.Sigmoid
                        nc.scalar.activation(
                            a[:, :, 0:Wd], pt[:, t, :, 0:Wd],
                            fn, bias=misc_t[:, 2 + t:3 + t])
                        acts[t] = a

                    # cell update for this m while the other m's gates run
                    t1 = tpool.tile([128, 2, ND], bf, tag=f"t1_{m}")
                    nc.vector.tensor_mul(t1[:, :, 0:Wd],
                                         acts[0 + m][:, :, 0:Wd],
                                         acts[4 + m][:, :, 0:Wd])
                    nc.vector.tensor_mul(cs[:, m, :, 0:Wd], cs[:, m, :, 0:Wd],
                                         acts[2 + m][:, :, 0:Wd])
                    nc.vector.tensor_add(cs[:, m, :, 0:Wd], cs[:, m, :, 0:Wd],
                                         t1[:, :, 0:Wd])
                    t2 = tpool.tile([128, 2, ND], bf, tag=f"t2_{m}")
                    nc.scalar.activation(t2[:, :, 0:Wd], cs[:, m, :, 0:Wd],
                                         AF.Tanh)
                    nc.vector.tensor_mul(h[:, m, :, 0:Wd],
                                         acts[6 + m][:, :, 0:Wd],
                                         t2[:, :, 0:Wd])
                nc.vector.tensor_scalar_mul(
                    h[:, :, :, OWN:OWN + 1], h[:, :, :, OWN:OWN + 1],
                    misc_t[:, 10:11])

                nc.sync.dma_start(hs_out[d], h[:, :, :, 0:OWN])

    nc.finalize()
    return nc


def _get_compiled(nsteps=WD):
    if nsteps not in _COMPILED:
        _COMPILED[nsteps] = _build(nsteps)
    return _COMPILED[nsteps]


# ------------------------------------------------------------------- driver

def kernel(x, w_i2s, b_i2s, w_ih, b_ih, b_hh, k0, k1, b_s2s):
    from concourse.bass_utils import run_bass_kernel_spmd

    in_maps = _prep_inputs(x, w_i2s, b_i2s, w_ih, b_ih, b_hh, k0, k1, b_s2s)
    nc = _get_compiled()
    res = run_bass_kernel_spmd(nc, in_maps, list(range(NCORES)))
    return _assemble([res.results[c]["hs"] for c in range(NCORES)])
